# revision 37
# baseline (speedup 1.0000x reference)
"""
Distributed Trainium2 Bass kernel for the AttentionFactorLayer problem.

Math (reference):
    x_emb  = features @ W_K.T                     [N, E]
    scores = mask(Q @ x_emb.T / sqrt(E))          [K, N]
    W      = softmax(scores, axis=-1)             [K, N]
    fr     = W @ returns                          [K, 1]
    pred   = W.T @ fr                             [N, 1]
    gram   = W @ W.T + ridge*I                    [K, K]
    inv    = gram^-1
    beta_t = W.T @ inv                            [N, K]
    omega  = I_N - beta_t @ W                     [N, N]   <- 256 MB, the roofline
    resid  = omega @ returns = returns - beta_t @ fr

Distribution: N sharded over 8 cores (1024 omega rows each); inputs are small
so softmax statistics and the Newton-Schulz 64x64 inverse are computed
redundantly; the [65,65] gram partial sums are AllReduced (only collective).
Device factorization: scores = (Q/sqrt(E) @ W_K) @ features.T + maskrow
(weight product folded on host), E = exp(scores - max), r = 1/rowsum(E),
C = (r r^T) * inv; omega block = I - E[:,rows]^T @ (C @ E).  The I-diagonal
is injected via a per-core 0/1 "strip" input indexed with static offsets, so
the SPMD program is identical on every core (only input values differ).
bf16 is used for the large matmuls (scores, C@E, omega); softmax statistics,
gram, and the inverse stay f32.
"""

import numpy as np

import concourse.bass as bass
import concourse.mybir as mybir
from concourse.tile import TileContext
from concourse.bass_utils import run_bass_kernel_spmd

F32 = mybir.dt.float32
BF16 = mybir.dt.bfloat16
AF = mybir.ActivationFunctionType
OP = mybir.AluOpType
AX = mybir.AxisListType

NCORES = 8
N, F, K, E = 8192, 128, 64, 64
SH = N // NCORES            # 1024 rows of omega per core
SOFF = 1024                 # strip window offset
STRW = N + SOFF             # 9216 strip width
NEWTON = 7
RIDGE = 0.001
NEG = -1.0e9

_nc_cache = {}


def _split_multiwaits(nc):
    """Walrus only encodes one sync wait per instruction; peel extras onto
    same-engine NoOps inserted right before (engine blocks on each in order)."""
    for f in nc.m.functions:
        for blk in f.blocks:
            ins_at = []
            for idx, inst in enumerate(blk.instructions):
                si = inst.sync_info
                if si is None or len(si.on_wait) <= 1:
                    continue
                waits = list(si.on_wait)
                nops = []
                for w in waits[:-1]:
                    nop = mybir.InstNoOp(name=f"I-mw{nc.next_id()}", ins=[], outs=[])
                    nop.engine = inst.engine
                    nop.sync_info = mybir.SyncInfo(on_wait=[w], on_update=[])
                    nops.append(nop)
                inst.sync_info = mybir.SyncInfo(
                    on_wait=[waits[-1]], on_update=list(si.on_update)
                )
                ins_at.append((idx, nops))
            for idx, nops in reversed(ins_at):
                for nop in reversed(nops):
                    blk.instructions.insert(idx, nop)
    return nc


def _build():
    nc = bass.Bass()
    dp = nc.declare_dram_parameter
    # inputs (host-prepped layouts; per-core values where noted)
    fTb = dp("fTb", [F, N], BF16, isOutput=False)       # features.T (bf16)
    fTsf = dp("fTsf", [F, SH], F32, isOutput=False)     # per-core col shard (f32)
    mt = dp("mt", [F, K], BF16, isOutput=False)         # (Q/sqrt(E) @ W_K).T
    mtf = dp("mtf", [F, K], F32, isOutput=False)        # same, f32
    ones1b = dp("ones1b", [1, K], BF16, isOutput=False)
    mrow = dp("mrow", [1, N], BF16, isOutput=False)     # 0 / -1e9 mask row
    mrowsf = dp("mrowsf", [1, SH], F32, isOutput=False)  # per-core shard (f32)
    rrows = dp("rrows", [1, SH], F32, isOutput=False)   # per-core returns row
    rsh = dp("rsh", [128, SH // 128], F32, isOutput=False)  # per-core returns
    stripi = dp("stripi", [128, STRW], BF16, isOutput=False)  # per-core diag strip
    ident = dp("ident", [128, 128], F32, isOutput=False)
    onesr = dp("onesr", [1, 128], F32, isOutput=False)
    onesc = dp("onesc", [128, 1], F32, isOutput=False)
    twoI = dp("twoI", [K, K], F32, isOutput=False)
    rdgI = dp("rdgI", [K, K], F32, isOutput=False)
    # outputs (per-core blocks; host concatenates)
    oblk = dp("oblk", [SH, N], F32, isOutput=True)
    wcol = dp("wcol", [K, SH], F32, isOutput=True)
    bcol = dp("bcol", [K, SH], F32, isOutput=True)
    pred = dp("pred", [128, SH // 128], F32, isOutput=True)
    resd = dp("resd", [128, SH // 128], F32, isOutput=True)
    frout = dp("frout", [K, 1], F32, isOutput=True)

    # collective bounce buffers for the gram AllReduce
    gin = nc.dram_tensor("gin", [K + 1, K + 1], F32)
    gout = nc.dram_tensor("gout", [K + 1, K + 1], F32, addr_space="Shared")

    NC512 = N // 512        # 16
    SC512 = SH // 512       # 2
    MCH = SH // 128         # 8 omega row chunks
    GT = SH // 128          # 8 gram transpose tiles (shard only)

    with TileContext(nc) as tc:
        with (
            tc.tile_pool(name="big", bufs=2) as big,
            tc.tile_pool(name="bfp", bufs=1) as bfp,
            tc.tile_pool(name="stp", bufs=1) as stp,
            tc.tile_pool(name="sm", bufs=1) as sm,
            tc.tile_pool(name="wrk", bufs=3) as wrk,
            tc.tile_pool(name="pbig", bufs=3, space="PSUM") as pbig,
            tc.tile_pool(name="pt", bufs=2, space="PSUM") as pt,
            tc.tile_pool(name="pg", bufs=1, space="PSUM") as pg,
            tc.tile_pool(name="ps", bufs=2, space="PSUM") as ps,
        ):
            # ---- strip: strip[j, w] = 1.0 iff w == SOFF + r0 + j ----
            strip = stp.tile([128, STRW], BF16, tag="strip")

            # ---- constants / small inputs ----
            mt_sb = sm.tile([F, K], BF16, tag="mt")
            mtf_sb = sm.tile([F, K], F32, tag="mtf")
            ones1_sb = sm.tile([1, K], BF16, tag="ones1")
            ident_sb = sm.tile([128, 128], F32, tag="ident")
            onesr_sb = sm.tile([1, 128], F32, tag="onesr")
            onesc_sb = sm.tile([128, 1], F32, tag="onesc")
            twoI_sb = sm.tile([K, K], F32, tag="twoI")
            rdgI_sb = sm.tile([K, K], F32, tag="rdgI")
            rsh_sb = sm.tile([128, SH // 128], F32, tag="rsh")
            fTs_sb = sm.tile([F, SH], F32, tag="fTs")
            mrow_sb = sm.tile([1, N], BF16, tag="mrow")
            mrows_sb = sm.tile([1, SH], F32, tag="mrows")
            # shard-path inputs first: the f32 shard scores gate the gram
            nc.sync.dma_start(out=fTs_sb, in_=fTsf[:, :])
            nc.sync.dma_start(out=mtf_sb, in_=mtf[:, :])
            nc.sync.dma_start(out=mrows_sb, in_=mrowsf[:, :])
            nc.sync.dma_start(out=onesr_sb, in_=onesr[:, :])
            nc.sync.dma_start(out=ident_sb, in_=ident[:, :])
            nc.sync.dma_start(out=mt_sb, in_=mt[:, :])
            nc.sync.dma_start(out=ones1_sb, in_=ones1b[:, :])
            nc.sync.dma_start(out=mrow_sb, in_=mrow[:, :])
            nc.sync.dma_start(out=onesc_sb, in_=onesc[:, :])
            nc.sync.dma_start(out=twoI_sb, in_=twoI[:, :])
            nc.sync.dma_start(out=rdgI_sb, in_=rdgI[:, :])
            nc.sync.dma_start(out=rsh_sb, in_=rsh[:, :])

            # ---- load features.T (bf16, chunked so compute can start early) ----
            fT_sb = big.tile([F, N], BF16, tag="big")
            for q in range(4):
                nc.sync.dma_start(
                    out=fT_sb[:, q * (N // 4):(q + 1) * (N // 4)],
                    in_=fTb[:, q * (N // 4):(q + 1) * (N // 4)],
                )

            def scores_mm(lhs, ones_ap, rhs_f, rhs_m, c):
                pe = pbig.tile([K, 512], F32, tag="pbig", name=f"pe{nc.next_id()}")
                nc.tensor.matmul(
                    pe, lhs, rhs_f[:, c * 512:(c + 1) * 512], start=True, stop=False,
                )
                nc.tensor.matmul(
                    pe, ones_ap, rhs_m[:, c * 512:(c + 1) * 512],
                    start=False, stop=True, skip_group_check=True,
                )
                return pe

            # scores are O(5) here, so unshifted exp is safe and matches the
            # reference's shifted softmax to f32 rounding.
            # ---- shard scores+exp (f32): E65s = [exp(shard scores); returns row]
            E65s = sm.tile([K + 1, SH], F32, tag="E65s")
            nc.sync.dma_start(out=E65s[K:K + 1, :], in_=rrows[:, :])
            for c in range(SC512):
                pe = scores_mm(mtf_sb, onesr_sb[0:1, 0:K], fTs_sb, mrows_sb, c)
                nc.scalar.activation(
                    E65s[0:K, c * 512:(c + 1) * 512], pe, AF.Exp, bias=0.0,
                    scale=1.0,
                )
            Es = E65s[0:K, :]

            # ---- sharded gram: transpose my shard tiles, accumulate, AllReduce ----
            gtile = pg.tile([K + 1, K + 1], F32, tag="pg")
            for c in range(GT):
                ptile = pt.tile([128, K + 1], F32, tag="pt")
                nc.tensor.transpose(
                    ptile, E65s[:, c * 128:(c + 1) * 128],
                    ident_sb[0:K + 1, 0:K + 1],
                )
                tsb = wrk.tile([128, K + 1], F32, tag="tsb")
                if c % 2 == 0:
                    nc.vector.tensor_copy(tsb, ptile)
                else:
                    nc.scalar.copy(tsb, ptile)
                nc.tensor.matmul(
                    gtile, tsb, tsb, start=(c == 0), stop=(c == GT - 1),
                    skip_group_check=True,
                )
            Gpart = sm.tile([K + 1, K + 1], F32, tag="Gpart")
            nc.vector.tensor_copy(Gpart, gtile)
            nc.sync.dma_start(out=gin[:, :], in_=Gpart)
            nc.gpsimd.collective_compute(
                "AllReduce", OP.add,
                replica_groups=[list(range(NCORES))],
                ins=[gin[:, :]], outs=[gout[:, :]],
            )

            # ---- full scores+exp (bf16) — overlaps the AllReduce wait ----
            Eb = bfp.tile([K, N], BF16, tag="eb")
            psum16 = sm.tile([K, NC512], F32, tag="psum16")
            for c in range(NC512):
                pe = scores_mm(mt_sb, ones1_sb, fT_sb, mrow_sb, c)
                nc.scalar.activation(
                    Eb[:, c * 512:(c + 1) * 512], pe, AF.Exp, bias=0.0,
                    scale=1.0, accum_out=psum16[:, c:c + 1],
                )
            sumexp = sm.tile([K, 1], F32, tag="sumexp")
            nc.vector.reduce_sum(sumexp, psum16, axis=AX.X)
            Esb = bfp.tile([K, SH], BF16, tag="esb")
            nc.vector.tensor_copy(Esb, Es)

            rrec = sm.tile([K, 1], F32, tag="rrec")
            nc.vector.reciprocal(rrec, sumexp)
            negsum = sm.tile([K, 1], F32, tag="negsum")
            nc.vector.tensor_scalar_mul(negsum, sumexp, -1.0)
            rt = sm.tile([K + 1, 1], F32, tag="rt")
            nc.vector.tensor_copy(rt[0:K, :], rrec)
            nc.vector.tensor_copy(rt[K:K + 1, :], ident_sb[0:1, 0:1])
            # rt-derived broadcasts (ready before the AllReduce lands)
            ptT = ps.tile([1, K + 1], F32, tag="ps")
            nc.tensor.transpose(ptT, rt, ident_sb[0:K + 1, 0:K + 1])
            rtT_sb = sm.tile([1, K + 1], F32, tag="rtT")
            nc.vector.tensor_copy(rtT_sb, ptT)
            bcs = sm.tile([K + 1, K + 1], F32, tag="bcs")
            bc = ps.tile([K + 1, K + 1], F32, tag="ps")
            nc.tensor.matmul(bc, onesr_sb[0:1, 0:K + 1], rtT_sb)
            nc.vector.tensor_copy(bcs, bc)

            Gs = sm.tile([K + 1, K + 1], F32, tag="Gs")
            nc.sync.dma_start(out=Gs, in_=gout[:, :])

            # ---- scale gram by rt rt^T; A = gram + ridge I; fr ----
            rowsc = sm.tile([K + 1, K + 1], F32, tag="rowsc")
            nc.vector.tensor_scalar_mul(rowsc, Gs, rt)
            Asb = sm.tile([K + 1, K + 1], F32, tag="Asb")
            nc.vector.tensor_tensor(out=Asb, in0=rowsc, in1=bcs, op=OP.mult)
            nc.sync.dma_start(out=frout[:, :], in_=Asb[0:K, K:K + 1])
            Amat = sm.tile([K, K], F32, tag="Amat")
            nc.vector.tensor_tensor(
                out=Amat, in0=Asb[0:K, 0:K], in1=rdgI_sb, op=OP.add,
            )

            # ---- Newton-Schulz inverse, X0 = I / frobenius(A) ----
            sqt = wrk.tile([K, K], F32, tag="sqt")
            qsum = sm.tile([K, 1], F32, tag="qsum")
            nc.scalar.activation(sqt, Amat, AF.Square, accum_out=qsum)
            tp1 = ps.tile([1, 1], F32, tag="ps")
            nc.tensor.matmul(tp1, qsum, onesc_sb[0:K, 0:1])
            tns = sm.tile([1, 1], F32, tag="tns")
            nc.scalar.activation(tns, tp1, AF.Sqrt)
            c0 = sm.tile([1, 1], F32, tag="c0")
            nc.vector.reciprocal(c0, tns)
            bcp = ps.tile([K, 1], F32, tag="ps")
            nc.tensor.matmul(bcp, onesr_sb[0:1, 0:K], c0)
            c0b = sm.tile([K, 1], F32, tag="c0b")
            nc.vector.tensor_copy(c0b, bcp)
            X = wrk.tile([K, K], F32, tag="nx", name="X0")
            nc.vector.tensor_scalar_mul(X, ident_sb[0:K, 0:K], c0b)
            for it in range(NEWTON):
                axp = ps.tile([K, K], F32, tag="ps", name=f"axp{it}")
                nc.tensor.matmul(axp, Amat, X)
                Msb = wrk.tile([K, K], F32, tag="nm", name=f"nm{it}")
                nc.vector.tensor_tensor(out=Msb, in0=twoI_sb, in1=axp, op=OP.subtract)
                x2p = ps.tile([K, K], F32, tag="ps", name=f"x2p{it}")
                nc.tensor.matmul(x2p, X, Msb)
                Xn = wrk.tile([K, K], F32, tag="nx", name=f"X{it + 1}")
                nc.vector.tensor_copy(Xn, x2p)
                X = Xn

            # ---- Cneg = -(r r^T) * inv  (bf16 copy for the big matmuls) ----
            negr = sm.tile([K, 1], F32, tag="negr")
            nc.vector.tensor_scalar_mul(negr, rrec, -1.0)
            rsc2 = sm.tile([K, K], F32, tag="rsc2")
            nc.vector.tensor_scalar_mul(rsc2, X, negr)
            bcp2 = ps.tile([K, K], F32, tag="ps")
            nc.tensor.matmul(bcp2, onesr_sb[0:1, 0:K], rtT_sb[0:1, 0:K])
            Cneg32 = sm.tile([K, K], F32, tag="Cneg32")
            nc.vector.tensor_tensor(out=Cneg32, in0=rsc2, in1=bcp2, op=OP.mult)
            Cneg = sm.tile([K, K], BF16, tag="Cneg")
            nc.vector.tensor_copy(Cneg, Cneg32)

            # ---- CEn = (-C) @ E : full in bf16 ----
            CEn = bfp.tile([K, N], BF16, tag="cen")
            for c in range(NC512):
                pe = pbig.tile([K, 512], F32, tag="pbig")
                nc.tensor.matmul(pe, Cneg, Eb[:, c * 512:(c + 1) * 512])
                if c % 2 == 0:
                    nc.scalar.copy(CEn[:, c * 512:(c + 1) * 512], pe)
                else:
                    nc.vector.tensor_copy(CEn[:, c * 512:(c + 1) * 512], pe)

            # strip arrives from DRAM; only needed once omega starts
            nc.sync.dma_start(out=strip, in_=stripi[:, :])

            def small_outputs():
                CEns = sm.tile([K, SH], F32, tag="CEns")
                for c in range(SC512):
                    pe = pbig.tile([K, 512], F32, tag="pbig", name=f"pcs{c}")
                    nc.tensor.matmul(pe, Cneg32, Es[:, c * 512:(c + 1) * 512])
                    nc.scalar.copy(CEns[:, c * 512:(c + 1) * 512], pe)
                Wsh = sm.tile([K, SH], F32, tag="Wsh")
                nc.vector.tensor_scalar_mul(Wsh, Es, rrec)
                nc.sync.dma_start(out=wcol[:, :], in_=Wsh)
                Bsh = sm.tile([K, SH], F32, tag="Bsh")
                nc.vector.tensor_scalar_mul(Bsh, CEns, negsum)
                nc.sync.dma_start(out=bcol[:, :], in_=Bsh)
                fr_ap = Asb[0:K, K:K + 1]
                predsb = sm.tile([128, MCH], F32, tag="predsb")
                residsb = sm.tile([128, MCH], F32, tag="residsb")
                for a in range(MCH):
                    pp = ps.tile([128, 1], F32, tag="ps", name=f"pp{a}")
                    nc.tensor.matmul(pp, Wsh[:, a * 128:(a + 1) * 128], fr_ap)
                    nc.vector.tensor_copy(predsb[:, a:a + 1], pp)
                    rp = ps.tile([128, 1], F32, tag="ps", name=f"rp{a}")
                    nc.tensor.matmul(rp, Bsh[:, a * 128:(a + 1) * 128], fr_ap)
                    nc.vector.tensor_tensor(
                        out=residsb[:, a:a + 1], in0=rsh_sb[:, a:a + 1], in1=rp,
                        op=OP.subtract,
                    )
                nc.sync.dma_start(out=pred[:, :], in_=predsb)
                nc.sync.dma_start(out=resd[:, :], in_=residsb)

            # ---- omega row-chunks: I - Es^T @ CEn; 1MB DMA quarters ----
            for m in range(MCH):
                osb = big.tile([128, N], F32, tag="big", name=f"osb{m}")
                for c in range(NC512):
                    po = pbig.tile([128, 512], F32, tag="pbig", name=f"po{m}_{c}")
                    nc.tensor.matmul(
                        po, Esb[:, m * 128:(m + 1) * 128],
                        CEn[:, c * 512:(c + 1) * 512],
                    )
                    off = SOFF - m * 128 + c * 512
                    nc.vector.tensor_tensor(
                        out=osb[:, c * 512:(c + 1) * 512], in0=po,
                        in1=strip[:, off:off + 512], op=OP.add,
                    )
                    if c % 4 == 3:
                        q0 = (c - 3) * 512
                        nc.sync.dma_start(
                            out=oblk[m * 128:(m + 1) * 128, q0:q0 + 2048],
                            in_=osb[:, q0:q0 + 2048],
                        )
                if m == 0:
                    small_outputs()

    return nc


def _get_nc():
    if "nc" not in _nc_cache:
        _nc_cache["nc"] = _split_multiwaits(_build())
    return _nc_cache["nc"]


def _strip(r0):
    import ml_dtypes
    s = np.zeros((128, STRW), ml_dtypes.bfloat16)
    j = np.arange(128)
    s[j, SOFF + r0 + j] = 1.0
    return s


def _in_maps(features, returns, mask, W_K, Q):
    import ml_dtypes
    bf16 = ml_dtypes.bfloat16

    features = np.ascontiguousarray(np.asarray(features, dtype=np.float32))
    returns = np.ascontiguousarray(np.asarray(returns, dtype=np.float32))
    mask = np.asarray(mask)
    W_K = np.ascontiguousarray(np.asarray(W_K, dtype=np.float32))
    Q = np.ascontiguousarray(np.asarray(Q, dtype=np.float32))

    fT32 = np.ascontiguousarray(features.T)
    fT = fT32.astype(bf16)
    M = (Q / np.float32(np.sqrt(E))) @ W_K                   # [K, F] f32
    mtf = np.ascontiguousarray(M.T)                          # [F, K]
    mt = mtf.astype(bf16)
    ones1 = np.ones((1, K), bf16)
    mrow32 = np.where(mask > 0, np.float32(0.0), np.float32(NEG)).astype(
        np.float32
    ).reshape(1, N)
    mrow = mrow32.astype(bf16)
    rrow = returns.reshape(1, N)
    ident = np.eye(128, dtype=np.float32)
    onesr = np.ones((1, 128), np.float32)
    onesc = np.ones((128, 1), np.float32)
    twoI = (2.0 * np.eye(K)).astype(np.float32)
    rdgI = (RIDGE * np.eye(K)).astype(np.float32)

    in_maps = []
    for i in range(NCORES):
        r0 = i * SH
        in_maps.append({
            "fTb": fT,
            "fTsf": np.ascontiguousarray(fT32[:, r0:r0 + SH]),
            "mt": mt,
            "mtf": mtf,
            "ones1b": ones1,
            "mrow": mrow,
            "mrowsf": np.ascontiguousarray(mrow32[:, r0:r0 + SH]),
            "rrows": np.ascontiguousarray(rrow[:, r0:r0 + SH]),
            "rsh": np.ascontiguousarray(
                returns[r0:r0 + SH, 0].reshape(SH // 128, 128).T
            ),
            "stripi": _strip(r0),
            "ident": ident,
            "onesr": onesr,
            "onesc": onesc,
            "twoI": twoI,
            "rdgI": rdgI,
        })
    return in_maps


def _assemble(res):
    weights = np.concatenate([res[i]["wcol"] for i in range(NCORES)], axis=1)
    fr = res[0]["frout"]
    predicted = np.concatenate(
        [res[i]["pred"].T.reshape(SH, 1) for i in range(NCORES)], axis=0
    )
    residual = np.concatenate(
        [res[i]["resd"].T.reshape(SH, 1) for i in range(NCORES)], axis=0
    )
    omega = np.concatenate([res[i]["oblk"] for i in range(NCORES)], axis=0)
    beta = np.concatenate([res[i]["bcol"] for i in range(NCORES)], axis=1)
    return (weights, fr, predicted, residual, omega, beta)


def run(trace=False, tmpdir=None, **inputs):
    """Run on hardware; returns (outputs_tuple, BassKernelResults)."""
    nc = _get_nc()
    in_maps = _in_maps(**inputs)
    bkr = run_bass_kernel_spmd(
        nc, in_maps, core_ids=list(range(NCORES)), trace=trace, tmpdir=tmpdir,
    )
    return _assemble(bkr.results), bkr


def kernel(**inputs):
    outputs, _ = run(trace=False, **inputs)
    return outputs


# revision 40
# speedup vs baseline: 1.0927x; 1.0927x over previous
"""
Distributed Trainium2 Bass kernel for the AttentionFactorLayer problem.

Math (reference):
    x_emb  = features @ W_K.T                     [N, E]
    scores = mask(Q @ x_emb.T / sqrt(E))          [K, N]
    W      = softmax(scores, axis=-1)             [K, N]
    fr     = W @ returns                          [K, 1]
    pred   = W.T @ fr                             [N, 1]
    gram   = W @ W.T + ridge*I                    [K, K]
    inv    = gram^-1
    beta_t = W.T @ inv                            [N, K]
    omega  = I_N - beta_t @ W                     [N, N]   <- 256 MB, the roofline
    resid  = omega @ returns = returns - beta_t @ fr

Distribution: N sharded over 8 cores (1024 omega rows each); inputs are small
so softmax statistics and the Newton-Schulz 64x64 inverse are computed
redundantly; the [65,65] gram partial sums are AllReduced (only collective).
Device factorization: scores = (Q/sqrt(E) @ W_K) @ features.T + maskrow
(weight product folded on host), E = exp(scores - max), r = 1/rowsum(E),
C = (r r^T) * inv; omega block = I - E[:,rows]^T @ (C @ E).  The I-diagonal
is injected via a per-core 0/1 "strip" input indexed with static offsets, so
the SPMD program is identical on every core (only input values differ).
bf16 is used for the large matmuls (scores, C@E, omega); softmax statistics,
gram, and the inverse stay f32.
"""

import numpy as np

import concourse.bass as bass
import concourse.mybir as mybir
from concourse.tile import TileContext
from concourse.bass_utils import run_bass_kernel_spmd

F32 = mybir.dt.float32
BF16 = mybir.dt.bfloat16
AF = mybir.ActivationFunctionType
OP = mybir.AluOpType
AX = mybir.AxisListType

NCORES = 8
N, F, K, E = 8192, 128, 64, 64
SH = N // NCORES            # 1024 rows of omega per core
SOFF = 1024                 # strip window offset
STRW = N + SOFF             # 9216 strip width
NEWTON = 7
NREFINE = 5
RIDGE = 0.001
NEG = -1.0e9

_nc_cache = {}


def _split_multiwaits(nc):
    """Walrus only encodes one sync wait per instruction; peel extras onto
    same-engine NoOps inserted right before (engine blocks on each in order)."""
    for f in nc.m.functions:
        for blk in f.blocks:
            ins_at = []
            for idx, inst in enumerate(blk.instructions):
                si = inst.sync_info
                if si is None or len(si.on_wait) <= 1:
                    continue
                waits = list(si.on_wait)
                nops = []
                for w in waits[:-1]:
                    nop = mybir.InstNoOp(name=f"I-mw{nc.next_id()}", ins=[], outs=[])
                    nop.engine = inst.engine
                    nop.sync_info = mybir.SyncInfo(on_wait=[w], on_update=[])
                    nops.append(nop)
                inst.sync_info = mybir.SyncInfo(
                    on_wait=[waits[-1]], on_update=list(si.on_update)
                )
                ins_at.append((idx, nops))
            for idx, nops in reversed(ins_at):
                for nop in reversed(nops):
                    blk.instructions.insert(idx, nop)
    return nc


def _build():
    nc = bass.Bass()
    dp = nc.declare_dram_parameter
    # inputs (host-prepped layouts; per-core values where noted)
    fTb = dp("fTb", [F, N], BF16, isOutput=False)       # features.T (bf16)
    fTsf = dp("fTsf", [F, SH], F32, isOutput=False)     # per-core col shard (f32)
    mt = dp("mt", [F, K], BF16, isOutput=False)         # (Q/sqrt(E) @ W_K).T
    mtf = dp("mtf", [F, K], F32, isOutput=False)        # same, f32
    ones1b = dp("ones1b", [1, K], BF16, isOutput=False)
    mrow = dp("mrow", [1, N], BF16, isOutput=False)     # 0 / -1e9 mask row
    mrowsf = dp("mrowsf", [1, SH], F32, isOutput=False)  # per-core shard (f32)
    rrows = dp("rrows", [1, SH], F32, isOutput=False)   # per-core returns row
    rsh = dp("rsh", [128, SH // 128], F32, isOutput=False)  # per-core returns
    stripi = dp("stripi", [128, STRW], BF16, isOutput=False)  # per-core diag strip
    ident = dp("ident", [128, 128], F32, isOutput=False)
    onesr = dp("onesr", [1, 128], F32, isOutput=False)
    onesc = dp("onesc", [128, 1], F32, isOutput=False)
    twoI = dp("twoI", [K, K], F32, isOutput=False)
    rdgI = dp("rdgI", [K, K], F32, isOutput=False)
    # outputs (per-core blocks; host concatenates)
    oblk = dp("oblk", [SH, N], F32, isOutput=True)
    wcol = dp("wcol", [K, SH], F32, isOutput=True)
    bcol = dp("bcol", [K, SH], F32, isOutput=True)
    pred = dp("pred", [128, SH // 128], F32, isOutput=True)
    resd = dp("resd", [128, SH // 128], F32, isOutput=True)
    frout = dp("frout", [K, 1], F32, isOutput=True)

    # collective bounce buffers for the gram AllReduce
    gin = nc.dram_tensor("gin", [K + 1, K + 1], F32)
    gout = nc.dram_tensor("gout", [K + 1, K + 1], F32, addr_space="Shared")

    NC512 = N // 512        # 16
    SC512 = SH // 512       # 2
    MCH = SH // 128         # 8 omega row chunks
    GT = SH // 128          # 8 gram transpose tiles (shard only)

    with TileContext(nc) as tc:
        with (
            tc.tile_pool(name="big", bufs=2) as big,
            tc.tile_pool(name="bfp", bufs=1) as bfp,
            tc.tile_pool(name="stp", bufs=1) as stp,
            tc.tile_pool(name="sm", bufs=1) as sm,
            tc.tile_pool(name="wrk", bufs=3) as wrk,
            tc.tile_pool(name="pbig", bufs=3, space="PSUM") as pbig,
            tc.tile_pool(name="pt", bufs=2, space="PSUM") as pt,
            tc.tile_pool(name="pg", bufs=1, space="PSUM") as pg,
            tc.tile_pool(name="ps", bufs=2, space="PSUM") as ps,
        ):
            # ---- strip: strip[j, w] = 1.0 iff w == SOFF + r0 + j ----
            strip = stp.tile([128, STRW], BF16, tag="strip")

            # ---- constants / small inputs ----
            mt_sb = sm.tile([F, K], BF16, tag="mt")
            mtf_sb = sm.tile([F, K], F32, tag="mtf")
            ones1_sb = sm.tile([1, K], BF16, tag="ones1")
            ident_sb = sm.tile([128, 128], F32, tag="ident")
            onesr_sb = sm.tile([1, 128], F32, tag="onesr")
            onesc_sb = sm.tile([128, 1], F32, tag="onesc")
            twoI_sb = sm.tile([K, K], F32, tag="twoI")
            rdgI_sb = sm.tile([K, K], F32, tag="rdgI")
            rsh_sb = sm.tile([128, SH // 128], F32, tag="rsh")
            fTs_sb = sm.tile([F, SH], F32, tag="fTs")
            mrow_sb = sm.tile([1, N], BF16, tag="mrow")
            mrows_sb = sm.tile([1, SH], F32, tag="mrows")
            # shard-path inputs first: the f32 shard scores gate the gram
            nc.sync.dma_start(out=fTs_sb, in_=fTsf[:, :])
            nc.sync.dma_start(out=mtf_sb, in_=mtf[:, :])
            nc.sync.dma_start(out=mrows_sb, in_=mrowsf[:, :])
            nc.sync.dma_start(out=onesr_sb, in_=onesr[:, :])
            nc.sync.dma_start(out=ident_sb, in_=ident[:, :])
            nc.sync.dma_start(out=mt_sb, in_=mt[:, :])
            nc.sync.dma_start(out=ones1_sb, in_=ones1b[:, :])
            nc.sync.dma_start(out=mrow_sb, in_=mrow[:, :])
            nc.sync.dma_start(out=onesc_sb, in_=onesc[:, :])
            nc.sync.dma_start(out=twoI_sb, in_=twoI[:, :])
            nc.sync.dma_start(out=rdgI_sb, in_=rdgI[:, :])
            nc.sync.dma_start(out=rsh_sb, in_=rsh[:, :])

            # ---- load features.T (bf16, chunked so compute can start early) ----
            fT_sb = big.tile([F, N], BF16, tag="big")
            for q in range(4):
                nc.sync.dma_start(
                    out=fT_sb[:, q * (N // 4):(q + 1) * (N // 4)],
                    in_=fTb[:, q * (N // 4):(q + 1) * (N // 4)],
                )

            def scores_mm(lhs, ones_ap, rhs_f, rhs_m, c):
                pe = pbig.tile([K, 512], F32, tag="pbig", name=f"pe{nc.next_id()}")
                nc.tensor.matmul(
                    pe, lhs, rhs_f[:, c * 512:(c + 1) * 512], start=True, stop=False,
                )
                nc.tensor.matmul(
                    pe, ones_ap, rhs_m[:, c * 512:(c + 1) * 512],
                    start=False, stop=True, skip_group_check=True,
                )
                return pe

            # scores are O(5) here, so unshifted exp is safe and matches the
            # reference's shifted softmax to f32 rounding.
            # ---- shard scores+exp (f32): E65s = [exp(shard scores); returns row]
            E65s = sm.tile([K + 1, SH], F32, tag="E65s")
            nc.sync.dma_start(out=E65s[K:K + 1, :], in_=rrows[:, :])
            for c in range(SC512):
                pe = scores_mm(mtf_sb, onesr_sb[0:1, 0:K], fTs_sb, mrows_sb, c)
                nc.scalar.activation(
                    E65s[0:K, c * 512:(c + 1) * 512], pe, AF.Exp, bias=0.0,
                    scale=1.0,
                )
            Es = E65s[0:K, :]

            # ---- sharded gram: transpose my shard tiles, accumulate, AllReduce ----
            gtile = pg.tile([K + 1, K + 1], F32, tag="pg")
            for c in range(GT):
                ptile = pt.tile([128, K + 1], F32, tag="pt")
                nc.tensor.transpose(
                    ptile, E65s[:, c * 128:(c + 1) * 128],
                    ident_sb[0:K + 1, 0:K + 1],
                )
                tsb = wrk.tile([128, K + 1], F32, tag="tsb")
                if c % 2 == 0:
                    nc.vector.tensor_copy(tsb, ptile)
                else:
                    nc.scalar.copy(tsb, ptile)
                nc.tensor.matmul(
                    gtile, tsb, tsb, start=(c == 0), stop=(c == GT - 1),
                    skip_group_check=True,
                )
            Gpart = sm.tile([K + 1, K + 1], F32, tag="Gpart")
            nc.vector.tensor_copy(Gpart, gtile)
            nc.sync.dma_start(out=gin[:, :], in_=Gpart)
            nc.gpsimd.collective_compute(
                "AllReduce", OP.add,
                replica_groups=[list(range(NCORES))],
                ins=[gin[:, :]], outs=[gout[:, :]],
            )

            # ---- full scores+exp (bf16) — overlaps the AllReduce wait ----
            Eb = bfp.tile([K, N], BF16, tag="eb")
            psum16 = sm.tile([K, NC512], F32, tag="psum16")
            for c in range(NC512):
                pe = scores_mm(mt_sb, ones1_sb, fT_sb, mrow_sb, c)
                nc.scalar.activation(
                    Eb[:, c * 512:(c + 1) * 512], pe, AF.Exp, bias=0.0,
                    scale=1.0, accum_out=psum16[:, c:c + 1],
                )
            sumexp = sm.tile([K, 1], F32, tag="sumexp")
            nc.vector.reduce_sum(sumexp, psum16, axis=AX.X)
            Esb = bfp.tile([K, SH], BF16, tag="esb")
            nc.vector.tensor_copy(Esb, Es)

            rrec = sm.tile([K, 1], F32, tag="rrec")
            nc.vector.reciprocal(rrec, sumexp)
            negsum = sm.tile([K, 1], F32, tag="negsum")
            nc.vector.tensor_scalar_mul(negsum, sumexp, -1.0)
            rt = sm.tile([K + 1, 1], F32, tag="rt")
            nc.vector.tensor_copy(rt[0:K, :], rrec)
            nc.vector.tensor_copy(rt[K:K + 1, :], ident_sb[0:1, 0:1])
            # rt-derived broadcasts (ready before the AllReduce lands)
            ptT = ps.tile([1, K + 1], F32, tag="ps")
            nc.tensor.transpose(ptT, rt, ident_sb[0:K + 1, 0:K + 1])
            rtT_sb = sm.tile([1, K + 1], F32, tag="rtT")
            nc.vector.tensor_copy(rtT_sb, ptT)
            bcs = sm.tile([K + 1, K + 1], F32, tag="bcs")
            bc = ps.tile([K + 1, K + 1], F32, tag="ps")
            nc.tensor.matmul(bc, onesr_sb[0:1, 0:K + 1], rtT_sb)
            nc.vector.tensor_copy(bcs, bc)

            def newton_iters(Amat_ap, X, n, lbl):
                for it in range(n):
                    axp = ps.tile([K, K], F32, tag="ps", name=f"axp{lbl}{it}")
                    nc.tensor.matmul(axp, Amat_ap, X)
                    Msb = wrk.tile([K, K], F32, tag="nm", name=f"nm{lbl}{it}")
                    nc.vector.tensor_tensor(
                        out=Msb, in0=twoI_sb, in1=axp, op=OP.subtract
                    )
                    x2p = ps.tile([K, K], F32, tag="ps", name=f"x2p{lbl}{it}")
                    nc.tensor.matmul(x2p, X, Msb)
                    X = wrk.tile([K, K], F32, tag="nx", name=f"X{lbl}{it + 1}")
                    nc.vector.tensor_copy(X, x2p)
                return X

            # ---- warm-start Newton on 8x(local gram) under the AllReduce ----
            rowscL = sm.tile([K + 1, K + 1], F32, tag="rowscL")
            nc.vector.tensor_scalar(
                out=rowscL, in0=Gpart, scalar1=rt, scalar2=8.0,
                op0=OP.mult, op1=OP.mult,
            )
            AsbL = sm.tile([K + 1, K + 1], F32, tag="AsbL")
            nc.vector.tensor_tensor(out=AsbL, in0=rowscL, in1=bcs, op=OP.mult)
            AmatL = sm.tile([K, K], F32, tag="AmatL")
            nc.vector.tensor_tensor(
                out=AmatL, in0=AsbL[0:K, 0:K], in1=rdgI_sb, op=OP.add,
            )
            sqt = wrk.tile([K, K], F32, tag="sqt")
            qsum = sm.tile([K, 1], F32, tag="qsum")
            nc.scalar.activation(sqt, AmatL, AF.Square, accum_out=qsum)
            tp1 = ps.tile([1, 1], F32, tag="ps")
            nc.tensor.matmul(tp1, qsum, onesc_sb[0:K, 0:1])
            tns = sm.tile([1, 1], F32, tag="tns")
            nc.scalar.activation(tns, tp1, AF.Sqrt)
            c0 = sm.tile([1, 1], F32, tag="c0")
            nc.vector.reciprocal(c0, tns)
            bcp = ps.tile([K, 1], F32, tag="ps")
            nc.tensor.matmul(bcp, onesr_sb[0:1, 0:K], c0)
            c0b = sm.tile([K, 1], F32, tag="c0b")
            nc.vector.tensor_copy(c0b, bcp)
            X = wrk.tile([K, K], F32, tag="nx", name="X0")
            nc.vector.tensor_scalar_mul(X, ident_sb[0:K, 0:K], c0b)
            X = newton_iters(AmatL, X, NEWTON, "w")

            Gs = sm.tile([K + 1, K + 1], F32, tag="Gs")
            nc.sync.dma_start(out=Gs, in_=gout[:, :])

            # ---- true gram scale; A = gram + ridge I; fr; refine inverse ----
            rowsc = sm.tile([K + 1, K + 1], F32, tag="rowsc")
            nc.vector.tensor_scalar_mul(rowsc, Gs, rt)
            Asb = sm.tile([K + 1, K + 1], F32, tag="Asb")
            nc.vector.tensor_tensor(out=Asb, in0=rowsc, in1=bcs, op=OP.mult)
            nc.sync.dma_start(out=frout[:, :], in_=Asb[0:K, K:K + 1])
            Amat = sm.tile([K, K], F32, tag="Amat")
            nc.vector.tensor_tensor(
                out=Amat, in0=Asb[0:K, 0:K], in1=rdgI_sb, op=OP.add,
            )
            X = newton_iters(Amat, X, NREFINE, "r")

            # ---- Cneg = -(r r^T) * inv  (bf16 copy for the big matmuls) ----
            negr = sm.tile([K, 1], F32, tag="negr")
            nc.vector.tensor_scalar_mul(negr, rrec, -1.0)
            rsc2 = sm.tile([K, K], F32, tag="rsc2")
            nc.vector.tensor_scalar_mul(rsc2, X, negr)
            bcp2 = ps.tile([K, K], F32, tag="ps")
            nc.tensor.matmul(bcp2, onesr_sb[0:1, 0:K], rtT_sb[0:1, 0:K])
            Cneg32 = sm.tile([K, K], F32, tag="Cneg32")
            nc.vector.tensor_tensor(out=Cneg32, in0=rsc2, in1=bcp2, op=OP.mult)
            Cneg = sm.tile([K, K], BF16, tag="Cneg")
            nc.vector.tensor_copy(Cneg, Cneg32)

            # ---- CEn = (-C) @ E : full in bf16 ----
            CEn = bfp.tile([K, N], BF16, tag="cen")
            for c in range(NC512):
                pe = pbig.tile([K, 512], F32, tag="pbig")
                nc.tensor.matmul(pe, Cneg, Eb[:, c * 512:(c + 1) * 512])
                if c % 2 == 0:
                    nc.scalar.copy(CEn[:, c * 512:(c + 1) * 512], pe)
                else:
                    nc.vector.tensor_copy(CEn[:, c * 512:(c + 1) * 512], pe)

            # strip arrives from DRAM; only needed once omega starts
            nc.sync.dma_start(out=strip, in_=stripi[:, :])

            def small_outputs():
                CEns = sm.tile([K, SH], F32, tag="CEns")
                for c in range(SC512):
                    pe = pbig.tile([K, 512], F32, tag="pbig", name=f"pcs{c}")
                    nc.tensor.matmul(pe, Cneg32, Es[:, c * 512:(c + 1) * 512])
                    nc.scalar.copy(CEns[:, c * 512:(c + 1) * 512], pe)
                Wsh = sm.tile([K, SH], F32, tag="Wsh")
                nc.vector.tensor_scalar_mul(Wsh, Es, rrec)
                nc.sync.dma_start(out=wcol[:, :], in_=Wsh)
                Bsh = sm.tile([K, SH], F32, tag="Bsh")
                nc.vector.tensor_scalar_mul(Bsh, CEns, negsum)
                nc.sync.dma_start(out=bcol[:, :], in_=Bsh)
                fr_ap = Asb[0:K, K:K + 1]
                predsb = sm.tile([128, MCH], F32, tag="predsb")
                residsb = sm.tile([128, MCH], F32, tag="residsb")
                for a in range(MCH):
                    pp = ps.tile([128, 1], F32, tag="ps", name=f"pp{a}")
                    nc.tensor.matmul(pp, Wsh[:, a * 128:(a + 1) * 128], fr_ap)
                    nc.vector.tensor_copy(predsb[:, a:a + 1], pp)
                    rp = ps.tile([128, 1], F32, tag="ps", name=f"rp{a}")
                    nc.tensor.matmul(rp, Bsh[:, a * 128:(a + 1) * 128], fr_ap)
                    nc.vector.tensor_tensor(
                        out=residsb[:, a:a + 1], in0=rsh_sb[:, a:a + 1], in1=rp,
                        op=OP.subtract,
                    )
                nc.sync.dma_start(out=pred[:, :], in_=predsb)
                nc.sync.dma_start(out=resd[:, :], in_=residsb)

            # ---- omega row-chunks: I - Es^T @ CEn; 1MB DMA quarters ----
            for m in range(MCH):
                osb = big.tile([128, N], F32, tag="big", name=f"osb{m}")
                for c in range(NC512):
                    po = pbig.tile([128, 512], F32, tag="pbig", name=f"po{m}_{c}")
                    nc.tensor.matmul(
                        po, Esb[:, m * 128:(m + 1) * 128],
                        CEn[:, c * 512:(c + 1) * 512],
                    )
                    off = SOFF - m * 128 + c * 512
                    nc.vector.tensor_tensor(
                        out=osb[:, c * 512:(c + 1) * 512], in0=po,
                        in1=strip[:, off:off + 512], op=OP.add,
                    )
                    if c % 8 == 7:
                        q0 = (c - 7) * 512
                        nc.sync.dma_start(
                            out=oblk[m * 128:(m + 1) * 128, q0:q0 + 4096],
                            in_=osb[:, q0:q0 + 4096],
                        )
                if m == 0:
                    small_outputs()

    return nc


def _get_nc():
    if "nc" not in _nc_cache:
        _nc_cache["nc"] = _split_multiwaits(_build())
    return _nc_cache["nc"]


def _strip(r0):
    import ml_dtypes
    s = np.zeros((128, STRW), ml_dtypes.bfloat16)
    j = np.arange(128)
    s[j, SOFF + r0 + j] = 1.0
    return s


def _in_maps(features, returns, mask, W_K, Q):
    import ml_dtypes
    bf16 = ml_dtypes.bfloat16

    features = np.ascontiguousarray(np.asarray(features, dtype=np.float32))
    returns = np.ascontiguousarray(np.asarray(returns, dtype=np.float32))
    mask = np.asarray(mask)
    W_K = np.ascontiguousarray(np.asarray(W_K, dtype=np.float32))
    Q = np.ascontiguousarray(np.asarray(Q, dtype=np.float32))

    fT32 = np.ascontiguousarray(features.T)
    fT = fT32.astype(bf16)
    M = (Q / np.float32(np.sqrt(E))) @ W_K                   # [K, F] f32
    mtf = np.ascontiguousarray(M.T)                          # [F, K]
    mt = mtf.astype(bf16)
    ones1 = np.ones((1, K), bf16)
    mrow32 = np.where(mask > 0, np.float32(0.0), np.float32(NEG)).astype(
        np.float32
    ).reshape(1, N)
    mrow = mrow32.astype(bf16)
    rrow = returns.reshape(1, N)
    ident = np.eye(128, dtype=np.float32)
    onesr = np.ones((1, 128), np.float32)
    onesc = np.ones((128, 1), np.float32)
    twoI = (2.0 * np.eye(K)).astype(np.float32)
    rdgI = (RIDGE * np.eye(K)).astype(np.float32)

    in_maps = []
    for i in range(NCORES):
        r0 = i * SH
        in_maps.append({
            "fTb": fT,
            "fTsf": np.ascontiguousarray(fT32[:, r0:r0 + SH]),
            "mt": mt,
            "mtf": mtf,
            "ones1b": ones1,
            "mrow": mrow,
            "mrowsf": np.ascontiguousarray(mrow32[:, r0:r0 + SH]),
            "rrows": np.ascontiguousarray(rrow[:, r0:r0 + SH]),
            "rsh": np.ascontiguousarray(
                returns[r0:r0 + SH, 0].reshape(SH // 128, 128).T
            ),
            "stripi": _strip(r0),
            "ident": ident,
            "onesr": onesr,
            "onesc": onesc,
            "twoI": twoI,
            "rdgI": rdgI,
        })
    return in_maps


def _assemble(res):
    weights = np.concatenate([res[i]["wcol"] for i in range(NCORES)], axis=1)
    fr = res[0]["frout"]
    predicted = np.concatenate(
        [res[i]["pred"].T.reshape(SH, 1) for i in range(NCORES)], axis=0
    )
    residual = np.concatenate(
        [res[i]["resd"].T.reshape(SH, 1) for i in range(NCORES)], axis=0
    )
    omega = np.concatenate([res[i]["oblk"] for i in range(NCORES)], axis=0)
    beta = np.concatenate([res[i]["bcol"] for i in range(NCORES)], axis=1)
    return (weights, fr, predicted, residual, omega, beta)


def run(trace=False, tmpdir=None, **inputs):
    """Run on hardware; returns (outputs_tuple, BassKernelResults)."""
    nc = _get_nc()
    in_maps = _in_maps(**inputs)
    bkr = run_bass_kernel_spmd(
        nc, in_maps, core_ids=list(range(NCORES)), trace=trace, tmpdir=tmpdir,
    )
    return _assemble(bkr.results), bkr


def kernel(**inputs):
    outputs, _ = run(trace=False, **inputs)
    return outputs
